# revision 1
# baseline (speedup 1.0000x reference)
"""BoT tokenizer kernel for Trainium2 (Bass/Tile), 8-core data parallel.

The whole module (two small Linears + 23 single-feature Linears + token
concat) is algebraically one affine map:

    out[b, t*512:(t+1)*512] = x_perm[b, :] @ W_big[:, t*512:(t+1)*512] + bias_t

where x_perm is x with columns gathered into [fore(9), palm(7), single(23)]
order and W_big is block-sparse [39, 25*512]. Appending a ones column to
x_perm folds every bias into row 39 of W_big, so on-device the problem is
just 25 fp32 matmuls (K=40, N=512) per 128-row batch chunk, a PSUM->SBUF
copy each, and one big contiguous DMA per chunk. The kernel is output-DMA
bound: each core writes 1024*25*512*4 = 52.4 MB.
"""

import numpy as np

# sensor_map order constants (hardcoded from the model definition)
FORE_IDX = [0, 1, 2, 27, 28, 32, 33, 34, 38]
PALM_IDX = [4, 29, 30, 31, 35, 36, 37]
SINGLE_IDX = [3] + list(range(5, 27))

B = 8192
D = 512
T = 25
N_CORES = 8
B_LOC = B // N_CORES          # 1024 rows per core
K = 40                        # 39 permuted features + ones row
CHUNK = 128                   # batch rows per matmul tile
N_CHUNKS = B_LOC // CHUNK     # 8
ROW = T * D                   # 12800 f32 per batch row

_prog_cache = {}


def _build_program():
    """Build the 8-core SPMD Tile program (compiled once, cached)."""
    import concourse.bacc as bacc
    import concourse.mybir as mybir
    import concourse.tile as tile
    from concourse.bass import ts

    f32 = mybir.dt.float32

    nc = bacc.Bacc("TRN2", target_bir_lowering=False, debug=False,
                   num_devices=N_CORES)

    xT_d = nc.dram_tensor("xT", [K, B_LOC], f32, kind="ExternalInput")
    W_d = nc.dram_tensor("W", [K, ROW], f32, kind="ExternalInput")
    out_d = nc.dram_tensor("out", [B_LOC, ROW], f32, kind="ExternalOutput")

    with tile.TileContext(nc) as tc:
        with (
            tc.tile_pool(name="wp", bufs=1) as wp,
            tc.tile_pool(name="xp", bufs=1) as xp,
            tc.tile_pool(name="op", bufs=2) as op,
            tc.tile_pool(name="pp", bufs=8, space="PSUM") as pp,
        ):
            w_s = wp.tile([K, ROW], f32)
            nc.sync.dma_start(out=w_s[:], in_=W_d[:])
            x_s = xp.tile([K, B_LOC], f32)
            nc.sync.dma_start(out=x_s[:], in_=xT_d[:])

            for c in range(N_CHUNKS):
                o_t = op.tile([CHUNK, ROW], f32)
                for t in range(T):
                    p_t = pp.tile([CHUNK, D], f32)
                    nc.tensor.matmul(
                        p_t[:],
                        x_s[:, ts(c, CHUNK)],
                        w_s[:, ts(t, D)],
                        start=True,
                        stop=True,
                    )
                    eng = nc.vector if t % 2 == 0 else nc.scalar
                    if eng is nc.vector:
                        eng.tensor_copy(o_t[:, ts(t, D)], p_t[:])
                    else:
                        eng.copy(o_t[:, ts(t, D)], p_t[:])
                nc.sync.dma_start(out=out_d[ts(c, CHUNK), :], in_=o_t[:])

    nc.compile()
    return nc


def _host_prep(x, Wf, bf, Wp, bp, Ws, bs):
    """Gather/transpose x and assemble the block-sparse big weight."""
    order = np.array(FORE_IDX + PALM_IDX + SINGLE_IDX)
    xT = np.empty((K, B), dtype=np.float32)
    xT[:39] = x[:, order].T
    xT[39] = 1.0

    W = np.zeros((K, ROW), dtype=np.float32)
    # token 0: forehand 9-feature linear
    W[0:9, 0:D] = Wf.T
    # token 2: palm 7-feature linear (palm features live at rows 9..15)
    W[9:16, 2 * D:3 * D] = Wp.T
    # token 1 = single sensor 0 (wrist), tokens 3..24 = single sensors 1..22
    # single sensor k lives at row 16 + k
    tok_of_single = [1] + list(range(3, 25))
    for k in range(23):
        t = tok_of_single[k]
        W[16 + k, t * D:(t + 1) * D] = Ws[k]
    # bias row
    W[39, 0:D] = bf
    W[39, 2 * D:3 * D] = bp
    for k in range(23):
        t = tok_of_single[k]
        W[39, t * D:(t + 1) * D] = bs[k]
    return xT, W


def kernel(x, Wf, bf, Wp, bp, Ws, bs, _trace=False, _spmd_kwargs=None):
    from concourse.bass_utils import run_bass_kernel_spmd

    x = np.asarray(x, dtype=np.float32)
    xT, W = _host_prep(np.asarray(x, np.float32), np.asarray(Wf, np.float32),
                       np.asarray(bf, np.float32), np.asarray(Wp, np.float32),
                       np.asarray(bp, np.float32), np.asarray(Ws, np.float32),
                       np.asarray(bs, np.float32))

    if "nc" not in _prog_cache:
        _prog_cache["nc"] = _build_program()
    nc = _prog_cache["nc"]

    in_maps = [
        {
            "xT": np.ascontiguousarray(xT[:, i * B_LOC:(i + 1) * B_LOC]),
            "W": W,
        }
        for i in range(N_CORES)
    ]

    kwargs = dict(_spmd_kwargs or {})
    res = run_bass_kernel_spmd(nc, in_maps, core_ids=list(range(N_CORES)),
                               trace=_trace, **kwargs)
    out = np.concatenate([r["out"] for r in res.results], axis=0)
    if _trace:
        kernel.last_results = res
    return out.reshape(B, T, D)


# revision 4
# speedup vs baseline: 1.9533x; 1.9533x over previous
"""BoT tokenizer kernel for Trainium2 (Bass/Tile), 8-core data parallel.

Math: all 25 tokens are affine maps of x.
 - token 0 (fore) and token 2 (palm) are real (tiny-K) matmuls -> TensorE,
   with the bias folded in via a ones-row in the stationary operand.
 - the 23 single-feature tokens are rank-1: out = xs[b,k]*Ws[k,:] + bs[k,:].
   fp32 matmuls on the PE cost ~4 cyc/col, so these run instead as ONE fused
   scalar_tensor_tensor op each on VectorE/GpSimd:
       out = (Ws_rep * xs_scalar) + bs_rep
   against Ws/bs replicated across the 128 partitions (host-prepared).

The kernel is output-DMA bound: each core writes 1024*25*512*4 = 52.4 MB.
Output rows are assembled in SBUF [128 x 12800] so the HBM writes are fully
contiguous 3.1-3.4 MB blocks.
"""

import numpy as np

# sensor_map order constants (hardcoded from the model definition)
FORE_IDX = [0, 1, 2, 27, 28, 32, 33, 34, 38]
PALM_IDX = [4, 29, 30, 31, 35, 36, 37]
SINGLE_IDX = [3] + list(range(5, 27))

B = 8192
D = 512
T = 25
N_CORES = 8
B_LOC = B // N_CORES          # 1024 rows per core
KFP = 17                      # 9 fore + 7 palm + ones row
CHUNK = 128
N_CHUNKS = B_LOC // CHUNK     # 8
ROW = T * D                   # 12800 f32 per batch row
NS = 23                       # single-feature sensors
SINGLE_COLS = NS * D          # 11776

# token id for single sensor k
TOK_OF_SINGLE = [1] + list(range(3, 25))
# out-tile split: group A = tokens 0..11, group B = tokens 12..24
A_TOKS = 12
A_COLS = A_TOKS * D           # 6144
B_COLS = ROW - A_COLS         # 6656

_prog_cache = {}


def _build_program():
    import concourse.bacc as bacc
    import concourse.mybir as mybir
    import concourse.tile as tile
    from concourse.bass import ts

    f32 = mybir.dt.float32
    mult = mybir.AluOpType.mult
    add = mybir.AluOpType.add

    nc = bacc.Bacc("TRN2", target_bir_lowering=False, debug=False,
                   num_devices=N_CORES)

    xfpT_d = nc.dram_tensor("xfpT", [KFP, B_LOC], f32, kind="ExternalInput")
    wfp_d = nc.dram_tensor("Wfp", [KFP, 2 * D], f32, kind="ExternalInput")
    xsP_d = nc.dram_tensor("xsP", [CHUNK, N_CHUNKS * NS], f32,
                           kind="ExternalInput")
    wsr_d = nc.dram_tensor("wsrep", [CHUNK, SINGLE_COLS], f32,
                           kind="ExternalInput")
    bsr_d = nc.dram_tensor("bsrep", [CHUNK, SINGLE_COLS], f32,
                           kind="ExternalInput")
    out_d = nc.dram_tensor("out", [B_LOC, ROW], f32, kind="ExternalOutput")

    with tile.TileContext(nc) as tc:
        with (
            tc.tile_pool(name="cst", bufs=1) as cst,
            tc.tile_pool(name="rep", bufs=1) as rep,
            tc.tile_pool(name="op", bufs=2) as op,
            tc.tile_pool(name="pp", bufs=4, space="PSUM") as pp,
        ):
            xfp_s = cst.tile([KFP, B_LOC], f32)
            nc.sync.dma_start(out=xfp_s[:], in_=xfpT_d[:])
            wfp_s = cst.tile([KFP, 2 * D], f32)
            nc.sync.dma_start(out=wfp_s[:], in_=wfp_d[:])
            xsP_s = cst.tile([CHUNK, N_CHUNKS * NS], f32)
            nc.sync.dma_start(out=xsP_s[:], in_=xsP_d[:])

            wsr_s = rep.tile([CHUNK, SINGLE_COLS], f32)
            bsr_s = rep.tile([CHUNK, SINGLE_COLS], f32)
            # per-token DMAs so chunk-0 compute can start as slices land
            for k in range(NS):
                nc.sync.dma_start(out=wsr_s[:, ts(k, D)], in_=wsr_d[:, ts(k, D)])
                nc.sync.dma_start(out=bsr_s[:, ts(k, D)], in_=bsr_d[:, ts(k, D)])

            for c in range(N_CHUNKS):
                oA = op.tile([CHUNK, A_COLS], f32, tag="outA")
                oB = op.tile([CHUNK, B_COLS], f32, tag="outB")

                # fore (token 0) and palm (token 2) on the PE
                for t, col in ((0, 0), (2, 2 * D)):
                    p_t = pp.tile([CHUNK, D], f32)
                    nc.tensor.matmul(
                        p_t[:],
                        xfp_s[:, ts(c, CHUNK)],
                        wfp_s[:, ts(t // 2, D)],
                        start=True,
                        stop=True,
                    )
                    nc.scalar.copy(oA[:, col:col + D], p_t[:])

                # 23 single-feature tokens: one fused FMA each
                for k in range(NS):
                    t = TOK_OF_SINGLE[k]
                    if t < A_TOKS:
                        dst = oA[:, ts(t, D)]
                    else:
                        dst = oB[:, ts(t - A_TOKS, D)]
                    eng = nc.vector
                    eng.scalar_tensor_tensor(
                        dst,
                        wsr_s[:, ts(k, D)],
                        xsP_s[:, c * NS + k:c * NS + k + 1],
                        bsr_s[:, ts(k, D)],
                        mult,
                        add,
                    )

                nc.sync.dma_start(out=out_d[ts(c, CHUNK), 0:A_COLS], in_=oA[:])
                nc.sync.dma_start(out=out_d[ts(c, CHUNK), A_COLS:ROW], in_=oB[:])

    nc.compile()
    return nc


def _host_prep(x, Wf, bf, Wp, bp, Ws, bs):
    fore = np.asarray(FORE_IDX)
    palm = np.asarray(PALM_IDX)
    single = np.asarray(SINGLE_IDX)

    # [17, B]: fore feats, palm feats, ones
    xfpT = np.empty((KFP, B), dtype=np.float32)
    xfpT[0:9] = x[:, fore].T
    xfpT[9:16] = x[:, palm].T
    xfpT[16] = 1.0

    # PE rhs: cols 0:512 = fore token, cols 512:1024 = palm token
    wfp = np.zeros((KFP, 2 * D), dtype=np.float32)
    wfp[0:9, 0:D] = Wf.T
    wfp[16, 0:D] = bf
    wfp[9:16, D:2 * D] = Wp.T
    wfp[16, D:2 * D] = bp

    # per-partition scalars: xsP[p, c*23+k] = x[c*128+p, SINGLE_IDX[k]]
    xs = x[:, single]                                   # [B, 23]
    xsP = (xs.reshape(N_CORES, N_CHUNKS, CHUNK, NS)
             .transpose(0, 2, 1, 3)
             .reshape(N_CORES, CHUNK, N_CHUNKS * NS))
    xsP = np.ascontiguousarray(xsP)

    # replicated [128, 23*512] weight/bias images (shared by all cores)
    wsr = np.ascontiguousarray(
        np.broadcast_to(Ws.reshape(1, SINGLE_COLS), (CHUNK, SINGLE_COLS)))
    bsr = np.ascontiguousarray(
        np.broadcast_to(bs.reshape(1, SINGLE_COLS), (CHUNK, SINGLE_COLS)))
    return xfpT, wfp, xsP, wsr, bsr


def kernel(x, Wf, bf, Wp, bp, Ws, bs, _trace=False, _spmd_kwargs=None):
    from concourse.bass_utils import run_bass_kernel_spmd

    x = np.asarray(x, np.float32)
    xfpT, wfp, xsP, wsr, bsr = _host_prep(
        x, np.asarray(Wf, np.float32), np.asarray(bf, np.float32),
        np.asarray(Wp, np.float32), np.asarray(bp, np.float32),
        np.asarray(Ws, np.float32), np.asarray(bs, np.float32))

    if "nc" not in _prog_cache:
        _prog_cache["nc"] = _build_program()
    nc = _prog_cache["nc"]

    in_maps = [
        {
            "xfpT": np.ascontiguousarray(xfpT[:, i * B_LOC:(i + 1) * B_LOC]),
            "Wfp": wfp,
            "xsP": xsP[i],
            "wsrep": wsr,
            "bsrep": bsr,
        }
        for i in range(N_CORES)
    ]

    kwargs = dict(_spmd_kwargs or {})
    res = run_bass_kernel_spmd(nc, in_maps, core_ids=list(range(N_CORES)),
                               trace=_trace, **kwargs)
    out = np.concatenate([r["out"] for r in res.results], axis=0)
    if _trace:
        kernel.last_results = res
    return out.reshape(B, T, D)


# revision 6
# speedup vs baseline: 1.9768x; 1.0120x over previous
"""BoT tokenizer kernel for Trainium2 (Bass/Tile), 8-core data parallel.

Math: all 25 tokens are affine maps of x.
 - token 0 (fore) and token 2 (palm) are real (tiny-K) matmuls -> TensorE,
   with the bias folded in via a ones-row in the stationary operand.
 - the 23 single-feature tokens are rank-1: out = xs[b,k]*Ws[k,:] + bs[k,:].
   fp32 matmuls on the PE cost ~4 cyc/col, so these run instead as ONE fused
   scalar_tensor_tensor op each on VectorE/GpSimd:
       out = (Ws_rep * xs_scalar) + bs_rep
   against Ws/bs replicated across the 128 partitions (host-prepared).

The kernel is output-DMA bound: each core writes 1024*25*512*4 = 52.4 MB.
Output rows are assembled in SBUF [128 x 12800] so the HBM writes are fully
contiguous 3.1-3.4 MB blocks.
"""

import numpy as np

# sensor_map order constants (hardcoded from the model definition)
FORE_IDX = [0, 1, 2, 27, 28, 32, 33, 34, 38]
PALM_IDX = [4, 29, 30, 31, 35, 36, 37]
SINGLE_IDX = [3] + list(range(5, 27))

B = 8192
D = 512
T = 25
N_CORES = 8
B_LOC = B // N_CORES          # 1024 rows per core
KFP = 17                      # 9 fore + 7 palm + ones row
CHUNK = 128
N_CHUNKS = B_LOC // CHUNK     # 8
ROW = T * D                   # 12800 f32 per batch row
NS = 23                       # single-feature sensors
SINGLE_COLS = NS * D          # 11776

# token id for single sensor k
TOK_OF_SINGLE = [1] + list(range(3, 25))
# out-tile split: group A = tokens 0..11, group B = tokens 12..24
A_TOKS = 12
A_COLS = A_TOKS * D           # 6144
B_COLS = ROW - A_COLS         # 6656
# singles with k < N_SPLIT go ScalarE-mul + GpSimd-add; rest VectorE fused
N_SPLIT = 9

_prog_cache = {}


def _build_program():
    import concourse.bacc as bacc
    import concourse.mybir as mybir
    import concourse.tile as tile
    from concourse.bass import ts

    f32 = mybir.dt.float32
    mult = mybir.AluOpType.mult
    add = mybir.AluOpType.add

    nc = bacc.Bacc("TRN2", target_bir_lowering=False, debug=False,
                   num_devices=N_CORES)

    xfpT_d = nc.dram_tensor("xfpT", [KFP, B_LOC], f32, kind="ExternalInput")
    wfp_d = nc.dram_tensor("Wfp", [KFP, 2 * D], f32, kind="ExternalInput")
    xsP_d = nc.dram_tensor("xsP", [CHUNK, N_CHUNKS * NS], f32,
                           kind="ExternalInput")
    wsr_d = nc.dram_tensor("wsrep", [CHUNK, SINGLE_COLS], f32,
                           kind="ExternalInput")
    bsr_d = nc.dram_tensor("bsrep", [CHUNK, SINGLE_COLS], f32,
                           kind="ExternalInput")
    out_d = nc.dram_tensor("out", [B_LOC, ROW], f32, kind="ExternalOutput")

    with tile.TileContext(nc) as tc:
        with (
            tc.tile_pool(name="cst", bufs=1) as cst,
            tc.tile_pool(name="rep", bufs=1) as rep,
            tc.tile_pool(name="op", bufs=2) as op,
            tc.tile_pool(name="pp", bufs=4, space="PSUM") as pp,
        ):
            xfp_s = cst.tile([KFP, B_LOC], f32)
            nc.sync.dma_start(out=xfp_s[:], in_=xfpT_d[:])
            wfp_s = cst.tile([KFP, 2 * D], f32)
            nc.sync.dma_start(out=wfp_s[:], in_=wfp_d[:])
            xsP_s = cst.tile([CHUNK, N_CHUNKS * NS], f32)
            nc.sync.dma_start(out=xsP_s[:], in_=xsP_d[:])

            wsr_s = rep.tile([CHUNK, SINGLE_COLS], f32)
            bsr_s = rep.tile([CHUNK, SINGLE_COLS], f32)
            # per-token DMAs so chunk-0 compute can start as slices land
            for k in range(NS):
                nc.sync.dma_start(out=wsr_s[:, ts(k, D)], in_=wsr_d[:, ts(k, D)])
                nc.sync.dma_start(out=bsr_s[:, ts(k, D)], in_=bsr_d[:, ts(k, D)])

            for c in range(N_CHUNKS):
                oA = op.tile([CHUNK, A_COLS], f32, tag="outA")
                oB = op.tile([CHUNK, B_COLS], f32, tag="outB")

                # fore (token 0) and palm (token 2) on the PE
                for t, col in ((0, 0), (2, 2 * D)):
                    p_t = pp.tile([CHUNK, D], f32)
                    nc.tensor.matmul(
                        p_t[:],
                        xfp_s[:, ts(c, CHUNK)],
                        wfp_s[:, ts(t // 2, D)],
                        start=True,
                        stop=True,
                    )
                    nc.scalar.copy(oA[:, col:col + D], p_t[:])

                # 23 single-feature tokens. Two paths, balanced so no
                # engine exceeds the DMA roofline:
                #  - k < N_SPLIT: ScalarE mul (ACTIVATE scale=xs) then
                #    GpSimd in-place bias add
                #  - else: one fused scalar_tensor_tensor on VectorE
                for k in range(NS):
                    t = TOK_OF_SINGLE[k]
                    if t < A_TOKS:
                        dst = oA[:, ts(t, D)]
                    else:
                        dst = oB[:, ts(t - A_TOKS, D)]
                    xs_ap = xsP_s[:, c * NS + k:c * NS + k + 1]
                    if k < N_SPLIT:
                        nc.scalar.activation(
                            dst, wsr_s[:, ts(k, D)],
                            mybir.ActivationFunctionType.Copy, scale=xs_ap)
                        nc.gpsimd.tensor_tensor(
                            dst, dst, bsr_s[:, ts(k, D)], add)
                    else:
                        nc.vector.scalar_tensor_tensor(
                            dst,
                            wsr_s[:, ts(k, D)],
                            xs_ap,
                            bsr_s[:, ts(k, D)],
                            mult,
                            add,
                        )

                nc.sync.dma_start(out=out_d[ts(c, CHUNK), 0:A_COLS], in_=oA[:])
                nc.sync.dma_start(out=out_d[ts(c, CHUNK), A_COLS:ROW], in_=oB[:])

    nc.compile()
    return nc


def _host_prep(x, Wf, bf, Wp, bp, Ws, bs):
    fore = np.asarray(FORE_IDX)
    palm = np.asarray(PALM_IDX)
    single = np.asarray(SINGLE_IDX)

    # [17, B]: fore feats, palm feats, ones
    xfpT = np.empty((KFP, B), dtype=np.float32)
    xfpT[0:9] = x[:, fore].T
    xfpT[9:16] = x[:, palm].T
    xfpT[16] = 1.0

    # PE rhs: cols 0:512 = fore token, cols 512:1024 = palm token
    wfp = np.zeros((KFP, 2 * D), dtype=np.float32)
    wfp[0:9, 0:D] = Wf.T
    wfp[16, 0:D] = bf
    wfp[9:16, D:2 * D] = Wp.T
    wfp[16, D:2 * D] = bp

    # per-partition scalars: xsP[p, c*23+k] = x[c*128+p, SINGLE_IDX[k]]
    xs = x[:, single]                                   # [B, 23]
    xsP = (xs.reshape(N_CORES, N_CHUNKS, CHUNK, NS)
             .transpose(0, 2, 1, 3)
             .reshape(N_CORES, CHUNK, N_CHUNKS * NS))
    xsP = np.ascontiguousarray(xsP)

    # replicated [128, 23*512] weight/bias images (shared by all cores)
    wsr = np.ascontiguousarray(
        np.broadcast_to(Ws.reshape(1, SINGLE_COLS), (CHUNK, SINGLE_COLS)))
    bsr = np.ascontiguousarray(
        np.broadcast_to(bs.reshape(1, SINGLE_COLS), (CHUNK, SINGLE_COLS)))
    return xfpT, wfp, xsP, wsr, bsr


def kernel(x, Wf, bf, Wp, bp, Ws, bs, _trace=False, _spmd_kwargs=None):
    from concourse.bass_utils import run_bass_kernel_spmd

    x = np.asarray(x, np.float32)
    xfpT, wfp, xsP, wsr, bsr = _host_prep(
        x, np.asarray(Wf, np.float32), np.asarray(bf, np.float32),
        np.asarray(Wp, np.float32), np.asarray(bp, np.float32),
        np.asarray(Ws, np.float32), np.asarray(bs, np.float32))

    if "nc" not in _prog_cache:
        _prog_cache["nc"] = _build_program()
    nc = _prog_cache["nc"]

    in_maps = [
        {
            "xfpT": np.ascontiguousarray(xfpT[:, i * B_LOC:(i + 1) * B_LOC]),
            "Wfp": wfp,
            "xsP": xsP[i],
            "wsrep": wsr,
            "bsrep": bsr,
        }
        for i in range(N_CORES)
    ]

    kwargs = dict(_spmd_kwargs or {})
    res = run_bass_kernel_spmd(nc, in_maps, core_ids=list(range(N_CORES)),
                               trace=_trace, **kwargs)
    out = np.concatenate([r["out"] for r in res.results], axis=0)
    if _trace:
        kernel.last_results = res
    return out.reshape(B, T, D)


# revision 8
# speedup vs baseline: 2.0990x; 1.0618x over previous
"""BoT tokenizer kernel for Trainium2 (Bass/Tile), 8-core data parallel.

Math: all 25 tokens are affine maps of x.
 - token 0 (fore) and token 2 (palm) are real (tiny-K) matmuls -> TensorE,
   with the bias folded in via a ones-row in the stationary operand.
 - the 23 single-feature tokens are rank-1: out = xs[b,k]*Ws[k,:] + bs[k,:].
   fp32 matmuls on the PE cost ~4 cyc/col, so these run instead as ONE fused
   scalar_tensor_tensor op each on VectorE/GpSimd:
       out = (Ws_rep * xs_scalar) + bs_rep
   against Ws/bs replicated across the 128 partitions (host-prepared).

The kernel is output-DMA bound: each core writes 1024*25*512*4 = 52.4 MB.
Output rows are assembled in SBUF [128 x 12800] so the HBM writes are fully
contiguous 3.1-3.4 MB blocks.
"""

import numpy as np

# sensor_map order constants (hardcoded from the model definition)
FORE_IDX = [0, 1, 2, 27, 28, 32, 33, 34, 38]
PALM_IDX = [4, 29, 30, 31, 35, 36, 37]
SINGLE_IDX = [3] + list(range(5, 27))

B = 8192
D = 512
T = 25
N_CORES = 8
B_LOC = B // N_CORES          # 1024 rows per core
KFP = 17                      # 9 fore + 7 palm + ones row
CHUNK = 128
N_CHUNKS = B_LOC // CHUNK     # 8
ROW = T * D                   # 12800 f32 per batch row
NS = 23                       # single-feature sensors
SINGLE_COLS = NS * D          # 11776

# token id for single sensor k
TOK_OF_SINGLE = [1] + list(range(3, 25))
# out-tile split into 4 token groups for finer DMA pipelining
GROUPS = [(0, 6), (6, 12), (12, 19), (19, 25)]  # [t0, t1) token ranges

_prog_cache = {}


def _build_program():
    import concourse.bacc as bacc
    import concourse.mybir as mybir
    import concourse.tile as tile
    from concourse.bass import ts

    f32 = mybir.dt.float32
    mult = mybir.AluOpType.mult
    add = mybir.AluOpType.add

    nc = bacc.Bacc("TRN2", target_bir_lowering=False, debug=False,
                   num_devices=N_CORES)

    xfpT_d = nc.dram_tensor("xfpT", [KFP, B_LOC], f32, kind="ExternalInput")
    wfp_d = nc.dram_tensor("Wfp", [KFP, 2 * D], f32, kind="ExternalInput")
    xsP_d = nc.dram_tensor("xsP", [CHUNK, N_CHUNKS * NS], f32,
                           kind="ExternalInput")
    wsr_d = nc.dram_tensor("wsrep", [CHUNK, SINGLE_COLS], f32,
                           kind="ExternalInput")
    bsr_d = nc.dram_tensor("bsrep", [CHUNK, SINGLE_COLS], f32,
                           kind="ExternalInput")
    out_d = nc.dram_tensor("out", [B_LOC, ROW], f32, kind="ExternalOutput")

    with tile.TileContext(nc) as tc:
        with (
            tc.tile_pool(name="cst", bufs=1) as cst,
            tc.tile_pool(name="rep", bufs=1) as rep,
            tc.tile_pool(name="op", bufs=2) as op,
            tc.tile_pool(name="pp", bufs=4, space="PSUM") as pp,
        ):
            xfp_s = cst.tile([KFP, B_LOC], f32)
            nc.sync.dma_start(out=xfp_s[:], in_=xfpT_d[:])
            wfp_s = cst.tile([KFP, 2 * D], f32)
            nc.sync.dma_start(out=wfp_s[:], in_=wfp_d[:])
            xsP_s = cst.tile([CHUNK, N_CHUNKS * NS], f32)
            nc.sync.dma_start(out=xsP_s[:], in_=xsP_d[:])

            wsr_s = rep.tile([CHUNK, SINGLE_COLS], f32)
            bsr_s = rep.tile([CHUNK, SINGLE_COLS], f32)
            # per-token DMAs so chunk-0 compute can start as slices land
            for k in range(NS):
                nc.sync.dma_start(out=wsr_s[:, ts(k, D)], in_=wsr_d[:, ts(k, D)])
                nc.sync.dma_start(out=bsr_s[:, ts(k, D)], in_=bsr_d[:, ts(k, D)])

            # single-sensor k for token t: k = 0 if t == 1 else t - 2
            def k_of_tok(t):
                return 0 if t == 1 else t - 2

            for c in range(N_CHUNKS):
                # process token groups in DMA order so output DMAs start
                # as early as possible within each chunk
                for gi, (t0, t1) in enumerate(GROUPS):
                    cols = (t1 - t0) * D
                    o_t = op.tile([CHUNK, cols], f32, tag=f"out{gi}")
                    for t in range(t0, t1):
                        dst = o_t[:, ts(t - t0, D)]
                        if t in (0, 2):
                            # fore / palm linear on the PE, bias folded in
                            p_t = pp.tile([CHUNK, D], f32)
                            nc.tensor.matmul(
                                p_t[:],
                                xfp_s[:, ts(c, CHUNK)],
                                wfp_s[:, ts(t // 2, D)],
                                start=True,
                                stop=True,
                            )
                            nc.scalar.copy(dst, p_t[:])
                        else:
                            # rank-1 token: out = Ws[k]*xs + bs[k], one
                            # fused VectorE op
                            k = k_of_tok(t)
                            nc.vector.scalar_tensor_tensor(
                                dst,
                                wsr_s[:, ts(k, D)],
                                xsP_s[:, c * NS + k:c * NS + k + 1],
                                bsr_s[:, ts(k, D)],
                                mult,
                                add,
                            )
                    nc.sync.dma_start(
                        out=out_d[ts(c, CHUNK), t0 * D:t1 * D], in_=o_t[:])

    nc.compile()
    return nc


def _host_prep(x, Wf, bf, Wp, bp, Ws, bs):
    fore = np.asarray(FORE_IDX)
    palm = np.asarray(PALM_IDX)
    single = np.asarray(SINGLE_IDX)

    # [17, B]: fore feats, palm feats, ones
    xfpT = np.empty((KFP, B), dtype=np.float32)
    xfpT[0:9] = x[:, fore].T
    xfpT[9:16] = x[:, palm].T
    xfpT[16] = 1.0

    # PE rhs: cols 0:512 = fore token, cols 512:1024 = palm token
    wfp = np.zeros((KFP, 2 * D), dtype=np.float32)
    wfp[0:9, 0:D] = Wf.T
    wfp[16, 0:D] = bf
    wfp[9:16, D:2 * D] = Wp.T
    wfp[16, D:2 * D] = bp

    # per-partition scalars: xsP[p, c*23+k] = x[c*128+p, SINGLE_IDX[k]]
    xs = x[:, single]                                   # [B, 23]
    xsP = (xs.reshape(N_CORES, N_CHUNKS, CHUNK, NS)
             .transpose(0, 2, 1, 3)
             .reshape(N_CORES, CHUNK, N_CHUNKS * NS))
    xsP = np.ascontiguousarray(xsP)

    # replicated [128, 23*512] weight/bias images (shared by all cores)
    wsr = np.ascontiguousarray(
        np.broadcast_to(Ws.reshape(1, SINGLE_COLS), (CHUNK, SINGLE_COLS)))
    bsr = np.ascontiguousarray(
        np.broadcast_to(bs.reshape(1, SINGLE_COLS), (CHUNK, SINGLE_COLS)))
    return xfpT, wfp, xsP, wsr, bsr


def kernel(x, Wf, bf, Wp, bp, Ws, bs, _trace=False, _spmd_kwargs=None):
    from concourse.bass_utils import run_bass_kernel_spmd

    x = np.asarray(x, np.float32)
    xfpT, wfp, xsP, wsr, bsr = _host_prep(
        x, np.asarray(Wf, np.float32), np.asarray(bf, np.float32),
        np.asarray(Wp, np.float32), np.asarray(bp, np.float32),
        np.asarray(Ws, np.float32), np.asarray(bs, np.float32))

    if "nc" not in _prog_cache:
        _prog_cache["nc"] = _build_program()
    nc = _prog_cache["nc"]

    in_maps = [
        {
            "xfpT": np.ascontiguousarray(xfpT[:, i * B_LOC:(i + 1) * B_LOC]),
            "Wfp": wfp,
            "xsP": xsP[i],
            "wsrep": wsr,
            "bsrep": bsr,
        }
        for i in range(N_CORES)
    ]

    kwargs = dict(_spmd_kwargs or {})
    res = run_bass_kernel_spmd(nc, in_maps, core_ids=list(range(N_CORES)),
                               trace=_trace, **kwargs)
    out = np.concatenate([r["out"] for r in res.results], axis=0)
    if _trace:
        kernel.last_results = res
    return out.reshape(B, T, D)


# revision 15
# speedup vs baseline: 2.4123x; 1.1493x over previous
"""BoT tokenizer kernel for Trainium2 (Bass/Tile), 8-core data parallel.

All 25 output tokens are computed on the TensorEngine as bf16 matmuls with
an exact fp32 -> 3x bf16 mantissa split (8+8+8 = 24 bits):

    x = a0 + a1 + a2 (each bf16, split exact by construction)
    x*w = sum_{i,j} ai*wj   (each bf16 product is exact in fp32)

 - single-feature token k: K=12 matmul (9 cross products + 3 bias rows
   against a ones column)
 - fore token: 9 features -> K = 9*9+3 = 84
 - palm token: 7 features -> K = 7*9+3 = 66

bf16 matmuls stream 1 col/cycle (vs 4 for fp32), so the PE produces each
[128,512] token tile in ~215ns. PSUM->SBUF copies are split between
VectorE and ScalarE. The kernel is then purely output-DMA bound:
each core writes 1024*25*512*4 = 52.4 MB of fp32 to HBM.
"""

import numpy as np

FORE_IDX = [0, 1, 2, 27, 28, 32, 33, 34, 38]
PALM_IDX = [4, 29, 30, 31, 35, 36, 37]
SINGLE_IDX = [3] + list(range(5, 27))

B = 8192
D = 512
T = 25
N_CORES = 8
B_LOC = B // N_CORES          # 1024 rows per core
CHUNK = 128
N_CHUNKS = B_LOC // CHUNK     # 8
ROW = T * D                   # 12800
NS = 23

# token id for single sensor k: k=0 -> token 1 (wrist), k>=1 -> token k+2
TOK_OF_SINGLE = [1] + list(range(3, 25))
# out-tile token groups for finer DMA pipelining
GROUPS = [(0, 6), (6, 12), (12, 19), (19, 25)]

KF = 9 * 9 + 3                # 84
KP = 7 * 9 + 3                # 66
KS = 12
# singles packed 3 per tile at 32-partition offsets (matmul base partition
# must be 0/32/64)
S_TILES = [(a, min(a + 3, NS)) for a in range(0, NS, 3)]
S_STRIDE = 32

_prog_cache = {}


def _k_of_tok(t):
    return 0 if t == 1 else t - 2


def _build_program():
    import concourse.bacc as bacc
    import concourse.mybir as mybir
    import concourse.tile as tile
    from concourse.bass import ts

    f32 = mybir.dt.float32
    bf16 = mybir.dt.bfloat16

    nc = bacc.Bacc("TRN2", target_bir_lowering=False, debug=False,
                   num_devices=N_CORES)

    lf_d = nc.dram_tensor("lf", [KF, B_LOC], bf16, kind="ExternalInput")
    lp_d = nc.dram_tensor("lp", [KP, B_LOC], bf16, kind="ExternalInput")
    rf_d = nc.dram_tensor("rf", [KF, D], bf16, kind="ExternalInput")
    rp_d = nc.dram_tensor("rp", [KP, D], bf16, kind="ExternalInput")
    ls_d = [nc.dram_tensor(f"ls{i}", [(b - a) * S_STRIDE, B_LOC], bf16,
                           kind="ExternalInput")
            for i, (a, b) in enumerate(S_TILES)]
    rs_d = [nc.dram_tensor(f"rs{i}", [(b - a) * S_STRIDE, D], bf16,
                           kind="ExternalInput")
            for i, (a, b) in enumerate(S_TILES)]
    out_d = nc.dram_tensor("out", [B_LOC, ROW], f32, kind="ExternalOutput")

    with tile.TileContext(nc) as tc:
        with (
            tc.tile_pool(name="cst", bufs=1) as cst,
            tc.tile_pool(name="op", bufs=3) as op,
            tc.tile_pool(name="pp", bufs=8, space="PSUM") as pp,
        ):
            lf_s = cst.tile([KF, B_LOC], bf16)
            nc.sync.dma_start(out=lf_s[:], in_=lf_d[:])
            lp_s = cst.tile([KP, B_LOC], bf16)
            nc.sync.dma_start(out=lp_s[:], in_=lp_d[:])
            rf_s = cst.tile([KF, D], bf16)
            nc.sync.dma_start(out=rf_s[:], in_=rf_d[:])
            rp_s = cst.tile([KP, D], bf16)
            nc.sync.dma_start(out=rp_s[:], in_=rp_d[:])
            ls_s, rs_s = [], []
            for i, (a, b) in enumerate(S_TILES):
                lt = cst.tile([(b - a) * S_STRIDE, B_LOC], bf16,
                              name=f"ls{i}_s")
                nc.sync.dma_start(out=lt[:], in_=ls_d[i][:])
                ls_s.append(lt)
                rt = cst.tile([(b - a) * S_STRIDE, D], bf16, name=f"rs{i}_s")
                nc.sync.dma_start(out=rt[:], in_=rs_d[i][:])
                rs_s.append(rt)

            for c in range(N_CHUNKS):
                ncopy = 0
                for gi, (t0, t1) in enumerate(GROUPS):
                    o_t = op.tile([CHUNK, (t1 - t0) * D], f32, tag=f"out{gi}")
                    for t in range(t0, t1):
                        dst = o_t[:, ts(t - t0, D)]
                        if t == 0:
                            lhsT = lf_s[:, ts(c, CHUNK)]
                            rhs = rf_s[:]
                        elif t == 2:
                            lhsT = lp_s[:, ts(c, CHUNK)]
                            rhs = rp_s[:]
                        else:
                            k = _k_of_tok(t)
                            i = k // 3
                            off = S_STRIDE * (k - S_TILES[i][0])
                            lhsT = ls_s[i][off:off + KS, ts(c, CHUNK)]
                            rhs = rs_s[i][off:off + KS, :]
                        p_t = pp.tile([CHUNK, D], f32)
                        nc.tensor.matmul(p_t[:], lhsT, rhs,
                                         start=True, stop=True)
                        if ncopy % 2 == 0:
                            nc.vector.tensor_copy(dst, p_t[:])
                        else:
                            nc.scalar.copy(dst, p_t[:])
                        ncopy += 1
                    nc.sync.dma_start(
                        out=out_d[ts(c, CHUNK), t0 * D:t1 * D], in_=o_t[:])

    nc.compile()
    return nc


def _split3(v):
    """Exact fp32 -> (bf16, bf16, bf16) mantissa split: v = s0+s1+s2."""
    import ml_dtypes
    bf = ml_dtypes.bfloat16
    v = np.asarray(v, np.float32)
    s0 = v.astype(bf)
    r1 = v - s0.astype(np.float32)
    s1 = r1.astype(bf)
    r2 = r1 - s1.astype(np.float32)
    s2 = r2.astype(bf)
    return s0, s1, s2


def _lhs_rows(xcols):
    """lhsT rows for a feature block: a0,a0,a0,a1,a1,a1,a2,a2,a2 per feat.

    xcols: [B, F] fp32 -> [9F, B] bf16"""
    import ml_dtypes
    Bn, F = xcols.shape
    s0, s1, s2 = _split3(xcols)          # each [B, F]
    out = np.empty((F, 9, Bn), dtype=ml_dtypes.bfloat16)
    for i, s in enumerate((s0, s1, s2)):
        out[:, 3 * i:3 * i + 3, :] = s.T[:, None, :]
    return out.reshape(9 * F, Bn)


def _rhs_rows(wcols):
    """rhs rows for a feature block: w0,w1,w2,w0,w1,w2,w0,w1,w2 per feat.

    wcols: [F, D] fp32 -> [9F, D] bf16"""
    import ml_dtypes
    F, Dn = wcols.shape
    s0, s1, s2 = _split3(wcols)
    out = np.empty((F, 3, 3, Dn), dtype=ml_dtypes.bfloat16)
    for j, s in enumerate((s0, s1, s2)):
        out[:, :, j, :] = s[:, None, :]
    return out.reshape(9 * F, Dn)


def _host_prep(x, Wf, bf_, Wp, bp, Ws, bs):
    import ml_dtypes
    bf16 = ml_dtypes.bfloat16

    ones3 = np.ones((3, B), dtype=bf16)

    def bias_rows(bias):
        b0, b1, b2 = _split3(bias)       # [D] each
        return np.stack([b0, b1, b2])    # [3, D]

    # fore: lhsT [84, B], rhs [84, D]
    lf = np.concatenate([_lhs_rows(x[:, FORE_IDX]), ones3])
    rf = np.concatenate([_rhs_rows(np.asarray(Wf.T)), bias_rows(bf_)])
    # palm: [66, *]
    lp = np.concatenate([_lhs_rows(x[:, PALM_IDX]), ones3])
    rp = np.concatenate([_rhs_rows(np.asarray(Wp.T)), bias_rows(bp)])

    # singles: per sensor a [12, *] block, padded to 32-partition slots
    ls_all = np.zeros((NS * S_STRIDE, B), dtype=bf16)
    rs_all = np.zeros((NS * S_STRIDE, D), dtype=bf16)
    xs = x[:, SINGLE_IDX]                # [B, 23]
    for k in range(NS):
        o = S_STRIDE * k
        ls_all[o:o + 9] = _lhs_rows(xs[:, k:k + 1])
        ls_all[o + 9:o + KS] = ones3
        rs_all[o:o + 9] = _rhs_rows(Ws[k:k + 1])
        rs_all[o + 9:o + KS] = bias_rows(bs[k])
    return lf, rf, lp, rp, ls_all, rs_all


def kernel(x, Wf, bf, Wp, bp, Ws, bs, _trace=False, _spmd_kwargs=None):
    from concourse.bass_utils import run_bass_kernel_spmd

    x = np.asarray(x, np.float32)
    lf, rf, lp, rp, ls_all, rs_all = _host_prep(
        x, np.asarray(Wf, np.float32), np.asarray(bf, np.float32),
        np.asarray(Wp, np.float32), np.asarray(bp, np.float32),
        np.asarray(Ws, np.float32), np.asarray(bs, np.float32))

    if "nc" not in _prog_cache:
        _prog_cache["nc"] = _build_program()
    nc = _prog_cache["nc"]

    in_maps = []
    for i in range(N_CORES):
        sl = slice(i * B_LOC, (i + 1) * B_LOC)
        m = {
            "lf": np.ascontiguousarray(lf[:, sl]),
            "lp": np.ascontiguousarray(lp[:, sl]),
            "rf": rf,
            "rp": rp,
        }
        for j, (a, b) in enumerate(S_TILES):
            m[f"ls{j}"] = np.ascontiguousarray(
                ls_all[S_STRIDE * a:S_STRIDE * b, sl])
            m[f"rs{j}"] = np.ascontiguousarray(rs_all[S_STRIDE * a:S_STRIDE * b])
        in_maps.append(m)

    kwargs = dict(_spmd_kwargs or {})
    res = run_bass_kernel_spmd(nc, in_maps, core_ids=list(range(N_CORES)),
                               trace=_trace, **kwargs)
    out = np.concatenate([r["out"] for r in res.results], axis=0)
    if _trace:
        kernel.last_results = res
    return out.reshape(B, T, D)
